# revision 9
# baseline (speedup 1.0000x reference)
"""GCN aggregator kernel for 8 TRN2 NeuronCores.

Computation (reference):
    f        = segment_mean(neigh, seg_ids)        # [N, 128]
    x_out    = (x + f) @ W.T                       # [N, 128]
    neigh_out= neigh @ W.T                         # [E, 128]

Sharding: pure data parallel over nodes. Core c gets nodes
[6250c, 6250(c+1)) and their contiguous edges [100000c, 100000(c+1)).
No cross-core communication.

Device algorithm (per core), memory-bound streaming:
  - neigh is streamed in groups of 2048 edges (16 tiles of 128 edges).
  - For each 128-edge tile T (natural layout [128 edge-part, 128 feat]):
      psA = T^T @ [I | A/16]   one bf16 matmul: cols 0:128 = T transposed
                               (feat-on-partition), cols 128:136 = per-node
                               neighbor means (A is the 0/1 edge->node map
                               prescaled by 1/deg; each 128-edge tile covers
                               exactly 8 nodes since deg=16).
      psB = (T^T)^T @ W.T      second bf16 matmul -> neigh_out tile, natural
                               layout, fp32 PSUM, DMA'd out.
  - x side per 128-node group: transpose x via matmul with identity, add the
    staged means, one more matmul by W.T -> x_out.
All matmuls bf16 (fp32 would run at 1/4 PE rate); I/O stays fp32.
"""

import numpy as np
import ml_dtypes

N_CORES = 8
N, DEG, F = 50000, 16, 128
E = N * DEG
NPC = N // N_CORES          # 6250 nodes per core
EPC = E // N_CORES          # 100000 edges per core
GN = 128                    # nodes per group
GE = GN * DEG               # 2048 edges per group
TILES = GE // 128           # 16 edge-tiles per full group
N_FULL_G = NPC // GN        # 48 full groups
TAIL_N = NPC - N_FULL_G * GN            # 106 nodes in tail group
TAIL_E = TAIL_N * DEG                   # 1696 edges
TAIL_T_FULL = TAIL_E // 128             # 13 full tiles
TAIL_E_REM = TAIL_E - TAIL_T_FULL * 128  # 32 leftover edges
GROUPS = N_FULL_G + 1
IAW = F + 8                 # 136: transposed tile + 8 segment-mean columns

BF16 = ml_dtypes.bfloat16

_CACHE = {}


def _build_program(debug_full_groups=None):
    import concourse.bass as bass
    from concourse import bacc, mybir
    from concourse.tile import TileContext

    f32 = mybir.dt.float32
    bf16 = mybir.dt.bfloat16
    ADD = mybir.AluOpType.add

    n_full_g = N_FULL_G if debug_full_groups is None else debug_full_groups
    groups = GROUPS if debug_full_groups is None else debug_full_groups
    nc = bacc.Bacc(None, target_bir_lowering=False)
    x_in = nc.declare_dram_parameter("x", [NPC, F], bf16, isOutput=False)
    ne_in = nc.declare_dram_parameter("neigh", [EPC, F], f32, isOutput=False)
    wt_in = nc.declare_dram_parameter("wt", [F, F], bf16, isOutput=False)
    ia_in = nc.declare_dram_parameter("ia", [F, IAW], bf16, isOutput=False)
    x_out = nc.declare_dram_parameter("x_out", [NPC, F], f32, isOutput=True)
    ne_out = nc.declare_dram_parameter("neigh_out", [EPC, F], f32, isOutput=True)

    # p-major views: edge/node index e = a*128 + p -> [p, a*128 + i]
    e_full = n_full_g * GE                    # 98304 edges in full groups
    e_t0 = e_full + TAIL_T_FULL * 128         # 99968: start of 32-edge remnant
    ne_r = ne_in[:e_full].rearrange("(a p) i -> p a i", p=128)
    ne_rt = ne_in[e_full:e_t0].rearrange("(a p) i -> p a i", p=128)
    no_r = ne_out[:e_full].rearrange("(a p) i -> p a i", p=128)
    no_rt = ne_out[e_full:e_t0].rearrange("(a p) i -> p a i", p=128)
    n_full = n_full_g * GN                    # 6144 nodes in full groups
    x_r = x_in[:n_full].rearrange("(a p) i -> p a i", p=128)
    xo_r = x_out[:n_full].rearrange("(a p) i -> p a i", p=128)

    XO_BATCH = 16   # groups of x_out batched into one 1 MiB store

    with TileContext(nc) as tc:
        with (
            tc.tile_pool(name="consts", bufs=1) as cpool,
            tc.tile_pool(name="xres", bufs=1) as xres_pool,
            tc.tile_pool(name="inp", bufs=3) as in_pool,
            tc.tile_pool(name="inbf", bufs=2) as bf_pool,
            tc.tile_pool(name="stag", bufs=2) as stag_pool,
            tc.tile_pool(name="outp", bufs=3) as out_pool,
            tc.tile_pool(name="xsmall", bufs=2) as xs_pool,
            tc.tile_pool(name="xout", bufs=2) as xo_pool,
            tc.tile_pool(name="psA", bufs=3, space="PSUM") as psA_pool,
            tc.tile_pool(name="psB", bufs=3, space="PSUM") as psB_pool,
            tc.tile_pool(name="psX", bufs=1, space="PSUM") as psX_pool,
        ):
            ia_sb = cpool.tile([F, IAW], bf16)
            nc.sync.dma_start(out=ia_sb, in_=ia_in[:])
            wt_sb = cpool.tile([F, F], bf16)
            nc.sync.dma_start(out=wt_sb, in_=wt_in[:])

            x_res = xres_pool.tile([128, n_full], bf16)
            nc.sync.dma_start(
                out=x_res.rearrange("p (a i) -> p a i", i=F), in_=x_r)
            if debug_full_groups is None:
                x_tail = xres_pool.tile([128, F], bf16)
                nc.gpsimd.memset(x_tail, 0.0)
                nc.sync.dma_start(
                    out=x_tail[:TAIL_N, :], in_=x_in[n_full:NPC, :])

            xo_sb = None
            for g in range(groups):
                full = g < n_full_g
                ntiles = TILES if full else (TAIL_T_FULL + 1)
                ncols = ntiles * 128

                # ---- load 2048-edge slab (fp32) and downconvert to bf16
                slab = in_pool.tile([128, GE], f32, tag="slab")
                if full:
                    nc.sync.dma_start(
                        out=slab.rearrange("p (a i) -> p a i", i=F),
                        in_=ne_r[:, g * TILES:(g + 1) * TILES, :])
                else:
                    nc.sync.dma_start(
                        out=slab[:, :TAIL_T_FULL * 128].rearrange(
                            "p (a i) -> p a i", i=F),
                        in_=ne_rt)
                    nc.gpsimd.memset(
                        slab[:, TAIL_T_FULL * 128:ncols], 0.0)
                    nc.sync.dma_start(
                        out=slab[:TAIL_E_REM, TAIL_T_FULL * 128:ncols],
                        in_=ne_in[e_t0:EPC, :])
                slab_bf = bf_pool.tile([128, GE], bf16, tag="slab_bf")
                nc.vector.tensor_copy(
                    out=slab_bf[:, :ncols], in_=slab[:, :ncols])

                # ---- pass 1: transpose + fused segment-mean matmuls
                stag = stag_pool.tile([128, TILES * IAW], bf16, tag="stag")
                for t in range(ntiles):
                    psA = psA_pool.tile([128, IAW], f32, tag="psA")
                    nc.tensor.matmul(
                        out=psA,
                        lhsT=slab_bf[:, t * 128:(t + 1) * 128],
                        rhs=ia_sb,
                        start=True, stop=True)
                    nc.vector.tensor_copy(
                        out=stag[:, t * IAW:(t + 1) * IAW], in_=psA)
                if not full:
                    nc.gpsimd.memset(stag[:, ntiles * IAW:], 0.0)

                # ---- pass 2: neigh_out = neigh @ W.T
                oslab = out_pool.tile([128, GE], f32, tag="oslab")
                for t in range(ntiles):
                    psB = psB_pool.tile([128, F], f32, tag="psB")
                    nc.tensor.matmul(
                        out=psB,
                        lhsT=stag[:, t * IAW:t * IAW + F],
                        rhs=wt_sb,
                        start=True, stop=True)
                    nc.scalar.copy(
                        out=oslab[:, t * 128:(t + 1) * 128], in_=psB)
                if full:
                    nc.sync.dma_start(
                        out=no_r[:, g * TILES:(g + 1) * TILES, :],
                        in_=oslab.rearrange("p (a i) -> p a i", i=F))
                else:
                    nc.sync.dma_start(
                        out=no_rt,
                        in_=oslab[:, :TAIL_T_FULL * 128].rearrange(
                            "p (a i) -> p a i", i=F))
                    nc.sync.dma_start(
                        out=ne_out[e_t0:EPC, :],
                        in_=oslab[:TAIL_E_REM, TAIL_T_FULL * 128:ncols])

                # ---- x side: x_out_g = (x_g + f_g) @ W.T
                x_lhsT = x_res[:, g * F:(g + 1) * F] if full else x_tail
                psC = psX_pool.tile([128, F], f32, tag="psC")
                nc.tensor.matmul(
                    out=psC, lhsT=x_lhsT, rhs=ia_sb[:, :F],
                    start=True, stop=True)
                xf_bf = xs_pool.tile([128, F], bf16, tag="xf_bf")
                nc.vector.tensor_tensor(
                    out=xf_bf.rearrange("p (t c) -> p t c", t=TILES),
                    in0=psC.rearrange("p (t c) -> p t c", t=TILES),
                    in1=stag.rearrange("p (t c) -> p t c", t=TILES)[:, :, F:],
                    op=ADD)
                psD = psX_pool.tile([128, F], f32, tag="psD")
                nc.tensor.matmul(
                    out=psD, lhsT=xf_bf, rhs=wt_sb, start=True, stop=True)

                if full:
                    if g % XO_BATCH == 0:
                        xo_sb = xo_pool.tile([128, XO_BATCH * F], f32, tag="xo")
                    nc.scalar.copy(
                        out=xo_sb[:, (g % XO_BATCH) * F:(g % XO_BATCH + 1) * F],
                        in_=psD)
                    if g % XO_BATCH == XO_BATCH - 1:
                        b = g // XO_BATCH
                        nc.sync.dma_start(
                            out=xo_r[:, b * XO_BATCH:(b + 1) * XO_BATCH, :],
                            in_=xo_sb.rearrange("p (a i) -> p a i", i=F))
                else:
                    xo_t = xo_pool.tile([128, F], f32, tag="xo_t")
                    nc.scalar.copy(out=xo_t, in_=psD)
                    nc.sync.dma_start(
                        out=x_out[n_full:NPC, :], in_=xo_t[:TAIL_N, :])
    nc.compile()
    return nc


def _host_constants(W):
    """Wt = W.T (bf16) and the fused [I | A/deg] matrix."""
    wt = np.ascontiguousarray(W.T).astype(BF16)
    ia = np.zeros((F, IAW), dtype=np.float32)
    ia[:F, :F] = np.eye(F, dtype=np.float32)
    for e in range(F):
        ia[e, F + e // DEG] = 1.0 / DEG
    return wt, ia.astype(BF16)


def _fallback(x, neigh, seg_ids, W):
    """General (non-uniform segments) host path; never hit for the
    deterministic reference inputs, kept for robustness."""
    n = x.shape[0]
    counts = np.bincount(seg_ids, minlength=n).astype(np.float32)
    sums = np.zeros((n, x.shape[1]), dtype=np.float32)
    np.add.at(sums, seg_ids, neigh)
    f = sums / np.maximum(counts, 1)[:, None]
    f = np.where(counts[:, None] > 0, f, np.nan)  # match 0/0 -> nan semantics
    return ((x + f) @ W.T).astype(np.float32), (neigh @ W.T).astype(np.float32)


def kernel(x, neigh, seg_ids, W):
    x = np.ascontiguousarray(np.asarray(x, dtype=np.float32))
    neigh = np.ascontiguousarray(np.asarray(neigh, dtype=np.float32))
    seg_ids = np.asarray(seg_ids, dtype=np.int32)
    W = np.ascontiguousarray(np.asarray(W, dtype=np.float32))

    expected_seg = (np.arange(E, dtype=np.int64) // DEG).astype(np.int32)
    if (x.shape != (N, F) or neigh.shape != (E, F)
            or not np.array_equal(seg_ids, expected_seg)):
        return _fallback(x, neigh, seg_ids, W)

    from concourse.bass_utils import run_bass_kernel_spmd

    if "nc" not in _CACHE:
        _CACHE["nc"] = _build_program()
    nc = _CACHE["nc"]

    wt, ia = _host_constants(W)
    x_bf = x.astype(BF16)
    in_maps = []
    for c in range(N_CORES):
        in_maps.append({
            "x": x_bf[c * NPC:(c + 1) * NPC],
            "neigh": neigh[c * EPC:(c + 1) * EPC],
            "wt": wt,
            "ia": ia,
        })
    res = run_bass_kernel_spmd(nc, in_maps, list(range(N_CORES)))
    results = res.results
    x_out = np.concatenate([results[c]["x_out"] for c in range(N_CORES)], axis=0)
    ne_out = np.concatenate(
        [results[c]["neigh_out"] for c in range(N_CORES)], axis=0)
    return x_out, ne_out


def run_traced(x, neigh, seg_ids, W):
    """Like kernel() but returns (outputs, exec_time_ns) using the profiler."""
    from concourse.bass_utils import run_bass_kernel_spmd
    if "nc" not in _CACHE:
        _CACHE["nc"] = _build_program()
    nc = _CACHE["nc"]
    wt, ia = _host_constants(np.asarray(W, dtype=np.float32))
    x = np.ascontiguousarray(np.asarray(x, dtype=np.float32))
    neigh = np.ascontiguousarray(np.asarray(neigh, dtype=np.float32))
    x_bf = x.astype(BF16)
    in_maps = []
    for c in range(N_CORES):
        in_maps.append({
            "x": x_bf[c * NPC:(c + 1) * NPC],
            "neigh": neigh[c * EPC:(c + 1) * EPC],
            "wt": wt,
            "ia": ia,
        })
    res = run_bass_kernel_spmd(
        nc, in_maps, list(range(N_CORES)), trace=True)
    results = res.results
    x_out = np.concatenate([results[c]["x_out"] for c in range(N_CORES)], axis=0)
    ne_out = np.concatenate(
        [results[c]["neigh_out"] for c in range(N_CORES)], axis=0)
    return (x_out, ne_out), res.exec_time_ns, res
